# revision 9
# baseline (speedup 1.0000x reference)
"""Dead-zone squared-error mean over N=33554432 elements, data-parallel on 8 NeuronCores.

reference:  diff = inputs - targets
            dz   = where(|diff| < 0.1, 0, diff)
            out  = mean(dz * dz)            (scalar float32)

Strategy: shard N across 8 cores (4,194,304 elements each).  Per core, stream
[128 x CHUNK] f32 tiles of both operands from HBM, compute
    d = x - t                 (DVE)
    s = d^2                   (ACT, Square)
    r = (s >= 0.01) * s       (DVE scalar_tensor_tensor, fused mask+mul)
with the per-partition running sum captured by the instruction's accum_out.
The final CHUNK is processed as NSPLIT small contiguous sub-tiles so the
post-DMA serial chain is short.  Each core returns a [128, NCOL] stats block;
the host sums the partials in float64 and divides by N.
"""

import numpy as np

import concourse.bacc as bacc
import concourse.mybir as mybir
import concourse.tile as tile
from concourse.alu_op_type import AluOpType
from concourse.bass_utils import run_bass_kernel_spmd

N = 33554432
NCORES = 8
PER_CORE = N // NCORES          # 4194304
P = 128
CHUNK = 2048                    # free elems per bulk tile -> 1 MiB per DMA
NT = PER_CORE // (P * CHUNK)    # 16 tile-slots per core
NB = NT - 1                     # bulk tiles
NSPLIT = 4                      # last tile-slot split into NSPLIT sub-tiles
TAILC = CHUNK // NSPLIT         # 512
NCOL = NB + NSPLIT              # stats columns
THRESH_SQ = 0.01                # (dead-zone 0.1)^2

F32 = mybir.dt.float32

_CACHE = {}


def _build_nc():
    nc = bacc.Bacc()
    xb = nc.dram_tensor("xb", [NB, P, CHUNK], F32, kind="ExternalInput")
    tb = nc.dram_tensor("tb", [NB, P, CHUNK], F32, kind="ExternalInput")
    xs = nc.dram_tensor("xs", [NSPLIT, P, TAILC], F32, kind="ExternalInput")
    ts = nc.dram_tensor("ts", [NSPLIT, P, TAILC], F32, kind="ExternalInput")
    out = nc.dram_tensor("out", [P, NCOL], F32, kind="ExternalOutput")

    with tile.TileContext(nc) as tc:
        with (
            tc.tile_pool(name="io", bufs=4) as io_pool,
            tc.tile_pool(name="tmp", bufs=3) as tmp_pool,
            tc.tile_pool(name="stats", bufs=1) as stats_pool,
        ):
            stats = stats_pool.tile([P, NCOL], F32)

            def process(x_ap, t_ap, c, col, tag):
                xt = io_pool.tile([P, c], F32, tag="x" + tag)
                tt = io_pool.tile([P, c], F32, tag="t" + tag)
                nc.sync.dma_start(out=xt[:], in_=x_ap)
                nc.sync.dma_start(out=tt[:], in_=t_ap)
                d = tmp_pool.tile([P, c], F32, tag="d" + tag)
                nc.vector.tensor_sub(d[:], xt[:], tt[:])
                s = tmp_pool.tile([P, c], F32, tag="s" + tag)
                nc.scalar.activation(s[:], d[:], mybir.ActivationFunctionType.Square)
                r = tmp_pool.tile([P, c], F32, tag="r" + tag)
                # r = (s >= 0.01) * s ; stats[:, col] = per-partition sum of r
                nc.vector.scalar_tensor_tensor(
                    out=r[:],
                    in0=s[:],
                    scalar=THRESH_SQ,
                    in1=s[:],
                    op0=AluOpType.is_ge,
                    op1=AluOpType.mult,
                    accum_out=stats[:, col : col + 1],
                )

            for i in range(NB):
                process(xb[i], tb[i], CHUNK, i, "b")
            for j in range(NSPLIT):
                process(xs[j], ts[j], TAILC, NB + j, "s")
            nc.sync.dma_start(out=out[:], in_=stats[:])
    nc.finalize()
    return nc


def _shard(a: np.ndarray):
    a = np.ascontiguousarray(a, dtype=np.float32).reshape(NCORES, PER_CORE)
    bulk = a[:, : NB * P * CHUNK].reshape(NCORES, NB, P, CHUNK)
    tail = a[:, NB * P * CHUNK :].reshape(NCORES, NSPLIT, P, TAILC)
    return bulk, tail


def kernel(inputs: np.ndarray, targets: np.ndarray) -> np.ndarray:
    xbulk, xtail = _shard(inputs)
    tbulk, ttail = _shard(targets)

    if "nc" not in _CACHE:
        _CACHE["nc"] = _build_nc()
    nc = _CACHE["nc"]

    in_maps = [
        {"xb": xbulk[c], "tb": tbulk[c], "xs": xtail[c], "ts": ttail[c]}
        for c in range(NCORES)
    ]
    res = run_bass_kernel_spmd(nc, in_maps, list(range(NCORES)))

    total = 0.0
    for r in res.results:
        total += r["out"].astype(np.float64).sum()
    return np.array(total / N, dtype=np.float32)


# revision 10
# speedup vs baseline: 1.1842x; 1.1842x over previous
"""Dead-zone squared-error mean over N=33554432 elements, data-parallel on 8 NeuronCores.

reference:  diff = inputs - targets
            dz   = where(|diff| < 0.1, 0, diff)
            out  = mean(dz * dz)            (scalar float32)

Strategy: shard N across 8 cores (4,194,304 elements each).  Per core, stream
[128 x CHUNK] f32 tiles of both operands from HBM, compute
    d = x - t                 (DVE)
    s = d^2                   (ACT, Square)
    r = (s >= 0.01) * s       (DVE scalar_tensor_tensor, fused mask+mul)
with the per-partition running sum captured by the instruction's accum_out.
The final CHUNK is processed as NSPLIT small contiguous sub-tiles so the
post-DMA serial chain is short.  Each core returns a [128, NCOL] stats block;
the host sums the partials in float64 and divides by N.
"""

import numpy as np

import concourse.bacc as bacc
import concourse.mybir as mybir
import concourse.tile as tile
from concourse.alu_op_type import AluOpType
from concourse.bass_utils import run_bass_kernel_spmd

N = 33554432
NCORES = 8
PER_CORE = N // NCORES          # 4194304
P = 128
CHUNK = 2048                    # free elems per bulk tile -> 1 MiB per DMA
NT = PER_CORE // (P * CHUNK)    # 16 tile-slots per core
NB = NT - 1                     # bulk tiles
NSPLIT = 4                      # last tile-slot split into NSPLIT sub-tiles
TAILC = CHUNK // NSPLIT         # 512
NCOL = NB + NSPLIT              # stats columns
THRESH_SQ = 0.01                # (dead-zone 0.1)^2

F32 = mybir.dt.float32

_CACHE = {}


def _build_nc():
    nc = bacc.Bacc()
    xb = nc.dram_tensor("xb", [NB, P, CHUNK], F32, kind="ExternalInput")
    tb = nc.dram_tensor("tb", [NB, P, CHUNK], F32, kind="ExternalInput")
    xs = nc.dram_tensor("xs", [NSPLIT, P, TAILC], F32, kind="ExternalInput")
    ts = nc.dram_tensor("ts", [NSPLIT, P, TAILC], F32, kind="ExternalInput")
    out = nc.dram_tensor("out", [P, NCOL], F32, kind="ExternalOutput")

    with tile.TileContext(nc) as tc:
        with (
            tc.tile_pool(name="io", bufs=3) as io_pool,
            tc.tile_pool(name="tmp", bufs=3) as tmp_pool,
            tc.tile_pool(name="stats", bufs=1) as stats_pool,
        ):
            stats = stats_pool.tile([P, NCOL], F32)

            def load_and_square(x_ap, t_ap, c, tag):
                xt = io_pool.tile([P, c], F32, tag="x" + tag)
                tt = io_pool.tile([P, c], F32, tag="t" + tag)
                nc.sync.dma_start(out=xt[:], in_=x_ap)
                nc.sync.dma_start(out=tt[:], in_=t_ap)
                d = tmp_pool.tile([P, c], F32, tag="d" + tag)
                nc.vector.tensor_sub(d[:], xt[:], tt[:])
                s = tmp_pool.tile([P, c], F32, tag="s" + tag)
                nc.scalar.activation(s[:], d[:], mybir.ActivationFunctionType.Square)
                return s

            def masked_accum(s, c, col, tag):
                r = tmp_pool.tile([P, c], F32, tag="r" + tag)
                # r = (s >= 0.01) * s ; stats[:, col] = per-partition sum of r
                nc.vector.scalar_tensor_tensor(
                    out=r[:],
                    in0=s[:],
                    scalar=THRESH_SQ,
                    in1=s[:],
                    op0=AluOpType.is_ge,
                    op1=AluOpType.mult,
                    accum_out=stats[:, col : col + 1],
                )

            # Emission is software-pipelined one tile deep: the masked
            # accumulate of tile i is emitted after the subtract of tile i+1,
            # so the in-order Vector engine never stalls on the cross-engine
            # square between its own two ops for one tile.
            work = [(xb[i], tb[i], CHUNK, "b") for i in range(NB)]
            work += [(xs[j], ts[j], TAILC, "s") for j in range(NSPLIT)]
            pending = None  # (s_tile, c, col, tag)
            for col, (x_ap, t_ap, c, tag) in enumerate(work):
                s = load_and_square(x_ap, t_ap, c, tag)
                if pending is not None:
                    masked_accum(*pending)
                pending = (s, c, col, tag)
            masked_accum(*pending)
            nc.sync.dma_start(out=out[:], in_=stats[:])
    nc.finalize()
    return nc


def _shard(a: np.ndarray):
    a = np.ascontiguousarray(a, dtype=np.float32).reshape(NCORES, PER_CORE)
    bulk = a[:, : NB * P * CHUNK].reshape(NCORES, NB, P, CHUNK)
    tail = a[:, NB * P * CHUNK :].reshape(NCORES, NSPLIT, P, TAILC)
    return bulk, tail


def kernel(inputs: np.ndarray, targets: np.ndarray) -> np.ndarray:
    xbulk, xtail = _shard(inputs)
    tbulk, ttail = _shard(targets)

    if "nc" not in _CACHE:
        _CACHE["nc"] = _build_nc()
    nc = _CACHE["nc"]

    in_maps = [
        {"xb": xbulk[c], "tb": tbulk[c], "xs": xtail[c], "ts": ttail[c]}
        for c in range(NCORES)
    ]
    res = run_bass_kernel_spmd(nc, in_maps, list(range(NCORES)))

    total = 0.0
    for r in res.results:
        total += r["out"].astype(np.float64).sum()
    return np.array(total / N, dtype=np.float32)
